# revision 18
# baseline (speedup 1.0000x reference)
"""GuidedAttentionLoss on 8 Trainium2 NeuronCores (Bass/Tile), v4: PE matmul.

loss = mean(guide * a^T) over [B=64, T=2048, N=512], where
  guide[b,t,k] = G_b[k, o_t],  G_b[k,o] = 1 - exp(-((k-o)/N_b)^2/(2 s^2)),
  o_t = floor(N_b/T_b * t), valid for t < T_b, k < N_b.

Key identity: G_b[k,o] = phi((k-o)/N_b) is a smooth Gaussian-type kernel on
a bounded domain, so it is numerically low rank: G_b ~= U_b V_b^T with
R ~ 10 (sigma_11/sigma_1 ~ 1e-10).  Then

  term_b = sum_{k,t} G_b[k,o_t] a[k,t]
         = sum_r sum_k U_b[k,r] * (sum_t V_b[o_t,r] a[k,t])
         = <U_b^T, Vt_b^T A_b^T>   with  Vt_b[t,r] = V_b[o_t,r].

The inner contraction over t is a PE matmul Z = Vt^T @ A^T (contract t on
partitions, 128 per step, fp8 DoubleRow = 256 per step at 0.5 cyc/row),
accumulated in PSUM [R, N_b].  The outer <U, Z> is one tiny DVE
scalar_tensor_tensor with accum_out per batch.  No exp on device at all.

Per core (8 batches, one per slot):
  * a^T staged fp8 [128(t), SUBT, N_s]; Vt staged fp8 [128(t), SUBT, R]
    (host-expanded V[o_t], zero rows for t >= T_b mask the t padding;
    zero U columns for k >= N_b mask the k padding).
  * V columns are pow2-scaled to ~[64,128) max-abs for fp8; U is refit
    against the quantized V by least squares on host (kills quantization
    bias), applied in bf16 on the DVE.
  * fp8 DoubleRow ISA ('s3_lw_dual_fp8_restrictions'): the k-pair free-dim
    step of both matmul APs must be 16-byte aligned -> R = 16, N_s % 16 == 0.
  * Engine model: DMA-bound; ~5.9 MB/core at ~620 GB/s effective ~ 9.5us
    measured steady state (PE ~5-9.5us, DVE ~4us, ACT 0).  Baseline v3
    (DVE slab kernel) measured 24.5us on the same rig.
  * 64 batches dealt into 8 slots x 8 cores by annealing + sweep on
    cost = SUBT_s * ceil16(N_s) (DMA bytes = PE work).
"""

import os

import numpy as np
import ml_dtypes

# DMA chunk in t-subtiles: ~26 chunk DMAs total. Finer (2) regresses on
# per-DMA dispatch overhead; coarser (8+) loses PE/DMA overlap.
_CHUNK = int(os.environ.get("K_CHUNK", "4"))
_ABUFS = int(os.environ.get("K_ABUFS", "4"))  # a-tile pool depth
_DMAONLY = os.environ.get("K_DMAONLY", "0") == "1"  # diag: no compute
_PE2X = os.environ.get("K_PE2X", "0") == "1"  # diag: duplicate matmuls
_NOSTT = os.environ.get("K_NOSTT", "0") == "1"  # diag: skip DVE stage
_ACTCOPY = os.environ.get("K_ACTCOPY", "0") == "1"  # PSUM->SBUF via ACT

B, N_MAX, T_MAX = 64, 512, 2048
SIGMA = 0.4
N_CORES = 8
PART = 128
R = 16  # rank of the guide factorization (16 = fp8 DoubleRow
        # LdWeights step alignment; rank 10 already exact)
F8 = ml_dtypes.float8_e4m3
BF16 = ml_dtypes.bfloat16

# engine model (per core): ns per unit
_PE_NS = 0.8333  # mid p-state cycle; full speed is 0.4167
_DVE_NS = 1.0416666
_DMA_BPNS = 360.0


def _offsets(Nb, Tb):
    """Per-t offset o_t with exact reference fp32 math."""
    t = np.arange(Tb, dtype=np.float32)
    ratio = np.float32(Nb) / np.float32(Tb)
    return np.floor(ratio * t).astype(np.int64)


_factor_cache: dict[tuple[int, int], tuple[np.ndarray, np.ndarray]] = {}


def _factors(Nb):
    """Low-rank factors of G[k,o] = 1 - exp(-((k-o)/Nb)^2/(2 sigma^2)).

    Returns (Ut [R, Nb] float32 refit, Vq [Nb, R] float8).  V columns are
    pow2-scaled into fp8 range; U is the least-squares refit of G against
    the quantized V, so fp8 quantization of V adds no bias.
    """
    key = (Nb, R)
    if key in _factor_cache:
        return _factor_cache[key]
    k = np.arange(Nb, dtype=np.float64)
    G = 1.0 - np.exp(-np.subtract.outer(k, k) ** 2 /
                     (2.0 * SIGMA * SIGMA * Nb * Nb))
    _, _, Vt_ = np.linalg.svd(G)
    r = min(R, Nb)
    V = Vt_[:r].T  # [Nb, r], unit columns
    sc = 2.0 ** np.floor(np.log2(64.0 / np.abs(V).max(axis=0)))
    Vq = (V * sc).astype(F8)
    Vd = Vq.astype(np.float64)
    Ut, *_ = np.linalg.lstsq(Vd.T @ Vd, Vd.T @ G.T, rcond=None)  # [r, Nb]
    if r < R:
        Ut = np.concatenate([Ut, np.zeros((R - r, Nb))], axis=0)
        Vq = np.concatenate([Vq, np.zeros((Nb, R - r), dtype=F8)], axis=1)
    out = (Ut.astype(np.float32), Vq)
    _factor_cache[key] = out
    return out


def _plan(input_lengths, target_lengths):
    """Assign batches to (slot, core) minimizing sum_s SUBT_s * N_s.

    Returns (slots, len_a, len_v, len_u); slot dicts have idxs, N_s, SUBT,
    base_a, base_v, base_u.
    """
    Ns = np.asarray(input_lengths, dtype=np.int64)
    Ts = np.asarray(target_lengths, dtype=np.int64)
    assert Ns.shape == (B,) and Ts.shape == (B,)
    SUBTs = -(-Ts // PART)
    n_slots = B // N_CORES

    def slot_cost(g):
        n16 = -(-int(max(Ns[i] for i in g)) // 16) * 16
        return int(max(SUBTs[i] for i in g)) * n16

    rng = np.random.default_rng(0)

    def sweep(groups):
        improved = True
        while improved:
            improved = False
            for s1 in range(n_slots):
                for s2 in range(s1 + 1, n_slots):
                    g1, g2 = groups[s1], groups[s2]
                    for i1 in range(N_CORES):
                        for i2 in range(N_CORES):
                            c0 = slot_cost(g1) + slot_cost(g2)
                            g1[i1], g2[i2] = g2[i2], g1[i1]
                            if slot_cost(g1) + slot_cost(g2) < c0:
                                improved = True
                            else:
                                g1[i1], g2[i2] = g2[i2], g1[i1]
        return groups

    def anneal(groups, iters=150000, T0=400.0, T1=0.5):
        groups = [list(g) for g in groups]
        costs = [slot_cost(g) for g in groups]
        cur = sum(costs)
        best, bestg = cur, [list(g) for g in groups]
        log_ratio = np.log(T1 / T0)
        u_rand = rng.random(iters)
        idx = rng.integers(0, 8, size=(iters, 4))
        for it in range(iters):
            s1, s2, i1, i2 = idx[it]
            if s1 == s2:
                continue
            T = T0 * np.exp(log_ratio * it / iters)
            g1, g2 = groups[s1], groups[s2]
            g1[i1], g2[i2] = g2[i2], g1[i1]
            c1, c2 = slot_cost(g1), slot_cost(g2)
            d = c1 + c2 - costs[s1] - costs[s2]
            if d <= 0 or u_rand[it] < np.exp(-d / T):
                costs[s1], costs[s2] = c1, c2
                cur += d
                if cur < best:
                    best, bestg = cur, [list(g) for g in groups]
            else:
                g1[i1], g2[i2] = g2[i2], g1[i1]
        return best, bestg

    order = np.argsort(-(SUBTs * 10000 + Ns))
    g0 = [list(order[s * N_CORES: (s + 1) * N_CORES]) for s in range(n_slots)]
    best_cost, best_g = anneal(g0, iters=200000)
    for _ in range(2):
        perm = rng.permutation(B)
        c, g = anneal([list(perm[s * N_CORES: (s + 1) * N_CORES])
                       for s in range(n_slots)], iters=120000)
        if c < best_cost:
            best_cost, best_g = c, g
    best_g = sweep([list(g) for g in best_g])

    # big slots first so the pipeline tail is short
    best_g.sort(key=lambda g: -slot_cost(g))
    slots, base_a, base_v, base_u = [], 0, 0, 0
    for g in best_g:
        idxs = np.array([int(i) for i in g])
        N_s = -(-int(Ns[idxs].max()) // 16) * 16  # DoubleRow pair-step % 16
        SUBT = int(SUBTs[idxs].max())
        slots.append(dict(idxs=idxs, N_s=N_s, SUBT=SUBT,
                          base_a=base_a, base_v=base_v, base_u=base_u))
        base_a += SUBT * N_s
        base_v += SUBT * R
        base_u += N_s
    return slots, base_a, base_v, base_u


def _host_inputs(alignments, input_lengths, target_lengths, slots,
                 len_a, len_v, len_u):
    """Per-core input dicts for run_bass_kernel_spmd."""
    alignments = np.asarray(alignments)
    in_maps = []
    for core in range(N_CORES):
        blob_a = np.zeros((PART, len_a), dtype=F8)
        blob_v = np.zeros((PART, len_v), dtype=F8)
        blob_u = np.zeros((R, len_u), dtype=BF16)
        for sl in slots:
            b = int(sl["idxs"][core])
            Nb = int(input_lengths[b])
            Tb = int(target_lengths[b])
            N_s, SUBT = sl["N_s"], sl["SUBT"]
            Ut, Vq = _factors(Nb)
            o_t = _offsets(Nb, Tb)

            aT = np.ascontiguousarray(
                alignments[b, :Nb, :Tb].T).astype(F8)  # [Tb, Nb]
            a3 = blob_a[:, sl["base_a"]: sl["base_a"] + SUBT * N_s]
            a3 = a3.reshape(PART, SUBT, N_s)
            full, rem = Tb // PART, Tb % PART
            if full:
                a3[:, :full, :Nb] = aT[: full * PART].reshape(
                    full, PART, Nb).transpose(1, 0, 2)
            if rem:
                a3[:rem, full, :Nb] = aT[full * PART:]

            Vexp = Vq[o_t]  # [Tb, R] fp8
            v3 = blob_v[:, sl["base_v"]: sl["base_v"] + SUBT * R]
            v3 = v3.reshape(PART, SUBT, R)
            if full:
                v3[:, :full, :] = Vexp[: full * PART].reshape(
                    full, PART, R).transpose(1, 0, 2)
            if rem:
                v3[:rem, full, :] = Vexp[full * PART:]

            blob_u[:, sl["base_u"]: sl["base_u"] + Nb] = Ut.astype(BF16)
        in_maps.append({"blob_a": blob_a, "blob_v": blob_v, "blob_u": blob_u})
    return in_maps


def _build_bass(slots, reps: int = 1):
    import concourse.bacc as bacc
    import concourse.mybir as mybir
    from concourse.tile import TileContext

    fp32 = mybir.dt.float32
    bf16 = mybir.dt.bfloat16
    f8 = mybir.dt.float8e4
    n_slots = len(slots)
    len_a = sum(sl["SUBT"] * sl["N_s"] for sl in slots)
    len_v = sum(sl["SUBT"] * R for sl in slots)
    len_u = sum(sl["N_s"] for sl in slots)
    max_a = max(sl["SUBT"] * sl["N_s"] for sl in slots)
    max_n = max(sl["N_s"] for sl in slots)

    nc = bacc.Bacc(
        "TRN2", target_bir_lowering=False, debug=False, num_devices=N_CORES
    )
    a_d = nc.dram_tensor("blob_a", [PART, len_a], f8, kind="ExternalInput")
    v_d = nc.dram_tensor("blob_v", [PART, len_v], f8, kind="ExternalInput")
    u_d = nc.dram_tensor("blob_u", [R, len_u], bf16, kind="ExternalInput")
    oacc_d = nc.dram_tensor("out_acc", [R, n_slots], fp32,
                            kind="ExternalOutput")

    with TileContext(nc) as tc:
        with (
            tc.tile_pool(name="const", bufs=1) as constp,
            tc.tile_pool(name="apool", bufs=_ABUFS) as apool,
            tc.tile_pool(name="mpool", bufs=4) as mpool,
            tc.psum_pool(name="zpool", bufs=4) as zpool,
        ):
            # constants go on the ACT HW-DGE queue so the a-stream can
            # start on the SP queue at t=0
            v_sb = constp.tile([PART, len_v], f8, tag="v")
            nc.scalar.dma_start(out=v_sb[:], in_=v_d.ap()[:])
            u_sb = constp.tile([R, len_u], bf16, tag="u")
            nc.scalar.dma_start(out=u_sb[:], in_=u_d.ap()[:])
            acc = constp.tile([R, n_slots], fp32, tag="acc")
            nc.vector.memset(acc[:], 0.0)

            for _rep in range(reps):
                for s, sl in enumerate(slots):
                    SUBT, N_s = sl["SUBT"], sl["N_s"]
                    a_t = apool.tile([PART, max_a], f8, tag="a")
                    a3 = a_t[:, : SUBT * N_s].rearrange(
                        "p (j n) -> p j n", j=SUBT)
                    # chunk the slot DMA at even-subtile boundaries so the
                    # first DoubleRow pairs can start as soon as their
                    # chunk lands, and chunks spread across DMA queues
                    sp = 0
                    while sp < SUBT:
                        ep = min(sp + _CHUNK, SUBT)
                        nc.sync.dma_start(
                            out=a_t[:, sp * N_s: ep * N_s],
                            in_=a_d.ap()[:, sl["base_a"] + sp * N_s:
                                         sl["base_a"] + ep * N_s],
                        )
                        sp = ep
                    if _DMAONLY:
                        continue
                    v3 = v_sb[:, sl["base_v"]: sl["base_v"] + SUBT * R
                              ].rearrange("p (j r) -> p j r", j=SUBT)
                    z_t = zpool.tile([R, max_n], fp32, tag="z")
                    npairs = SUBT // 2
                    for i in range(npairs):
                        nc.tensor.matmul(
                            z_t[:, :N_s],
                            v3[:, 2 * i: 2 * i + 2, :],
                            a3[:, 2 * i: 2 * i + 2, :],
                            start=(i == 0),
                            stop=(i == npairs - 1 and SUBT % 2 == 0),
                            perf_mode=mybir.MatmulPerfMode.DoubleRow,
                        )
                        if _PE2X:
                            zd = zpool.tile([R, max_n], fp32, tag="zdup")
                            nc.tensor.matmul(
                                zd[:, :N_s],
                                v3[:, 2 * i: 2 * i + 2, :],
                                a3[:, 2 * i: 2 * i + 2, :],
                                start=True, stop=True,
                                perf_mode=mybir.MatmulPerfMode.DoubleRow,
                            )
                    if SUBT % 2:
                        nc.tensor.matmul(
                            z_t[:, :N_s],
                            v3[:, SUBT - 1: SUBT, :],
                            a3[:, SUBT - 1: SUBT, :],
                            start=(npairs == 0),
                            stop=True,
                        )
                    if _NOSTT:
                        continue
                    if _ACTCOPY:
                        y_t = mpool.tile([R, max_n], bf16, tag="y")
                        nc.scalar.activation(
                            y_t[:, :N_s], z_t[:, :N_s],
                            mybir.ActivationFunctionType.Copy,
                        )
                        in0 = y_t[:, :N_s]
                    else:
                        in0 = z_t[:, :N_s]
                    m_t = mpool.tile([R, max_n], bf16, tag="m")
                    nc.vector.scalar_tensor_tensor(
                        out=m_t[:, :N_s],
                        in0=in0,
                        scalar=1.0,
                        in1=u_sb[:, sl["base_u"]: sl["base_u"] + N_s],
                        op0=mybir.AluOpType.mult,
                        op1=mybir.AluOpType.mult,
                        accum_out=acc[:, s: s + 1],
                    )
            nc.sync.dma_start(out=oacc_d.ap()[:], in_=acc[:])

    nc.compile()
    return nc


def _reduce_outputs(results):
    tot = 0.0
    for res in results:
        tot += np.asarray(res["out_acc"], dtype=np.float64).sum()
    return np.array(tot / float(B * N_MAX * T_MAX), dtype=np.float32)


def kernel(alignments, input_lengths, target_lengths):
    from concourse.bass_utils import run_bass_kernel_spmd

    slots, len_a, len_v, len_u = _plan(input_lengths, target_lengths)
    in_maps = _host_inputs(alignments, input_lengths, target_lengths, slots,
                           len_a, len_v, len_u)
    nc = _build_bass(slots, reps=1)
    out = run_bass_kernel_spmd(nc, in_maps, core_ids=list(range(N_CORES)))
    return _reduce_outputs(out.results)


if __name__ == "__main__":
    rng = np.random.default_rng(0)
    al = rng.random((B, N_MAX, T_MAX), dtype=np.float32)
    il = rng.integers(N_MAX // 2, N_MAX + 1, size=B).astype(np.int32)
    tl = rng.integers(T_MAX // 2, T_MAX + 1, size=B).astype(np.int32)
    print(kernel(alignments=al, input_lengths=il, target_lengths=tl))
